# revision 48
# baseline (speedup 1.0000x reference)
"""Trainium2 Bass kernel for nn_Attention_7584912245222.

Math (reference):
    hidden = tanh(memory @ Wh + (query @ Wq)[:, None, :])   # [B, T, D]
    s      = softmax(hidden @ v, axis=T)                    # [B, T]
    out    = einsum('btd,bt->bd', memory, s)                # [B, D]

Strategy: pure data-parallel over batch B=64 across 8 NeuronCores
(8 batches per core). Weights replicated. No collectives.

Device pipeline (per core, per batch b):
  - main GEMM in fp8 e4m3 with DoubleRow perf mode (2 contraction
    k-tiles per instruction, 2x PE throughput vs bf16). Computes
    hidden.T in [e(partitions), t(free)] orientation:
      lhsT = Wh[d,e] fp8 k-pair tiles (m-major layout, stationary)
      rhs  = mem[b].T fp8 tiles [d, t] (host pre-transposed, streamed)
    so the per-batch bias qvec[b][e] is a per-partition scalar, fused
    into the PSUM->SBUF tanh activation on ScalarE. The softmax
    averaging over T washes quantization noise out of the final
    weighted mean, keeping overall rel err ~1.8e-2 (< 2e-2 budget).
  - the bias qT = Wq.T @ query stays bf16 (its noise adds directly to
    the fp8 noise; bf16 keeps it negligible) and is computed per
    m-tile interleaved into batch 0 so the PE never waits for the Wq
    DMA.
  - v-weighting runs on the (otherwise idle) VectorE in fp16:
      acc_hv[p, t] += h_m[p, t] * v[m*128+p]   (scalar_tensor_tensor)
    and the partition reduction s[t] = sum_p acc_hv[p, t] is done with
    16 tiny PE matmuls producing s directly TRANSPOSED as [t, 1] cols.
  - softmax without max-subtraction (logits bounded, |s| < ~4).
  - final weighted sum on VectorE in fp16 (memN streamed as fp16):
      acc_d[p, d] += memN_j[p, d] * s_exp[j*128+p]
    followed by a 2-matmul partition reduction, scaled by 1/Z via the
    activation `scale` operand. The last batch uses PE matmuls
    pipelined per n-chunk instead, to shorten the kernel tail.

DMA rings: sync carries startup weights + memN(odd) + outputs,
gpsimd carries memT, vector carries memN(even) — keeps any single
ring under ~75 GB/s sustained so transfers never fall behind the PE.
"""

import sys

if "/opt/trn_rl_repo" not in sys.path:
    sys.path.insert(0, "/opt/trn_rl_repo")

import numpy as np
import ml_dtypes

import concourse.bass as bass
import concourse.tile as tile
from concourse import bacc, bass_isa, mybir
from concourse.bass_utils import run_bass_kernel_spmd

BF16 = ml_dtypes.bfloat16
FP16 = np.float16
E4M3 = ml_dtypes.float8_e4m3fn


def _install_ntff_hook_shim():
    """This image's antenv lacks axon_hooks; inject it so bass_utils'
    trace path (taken when BASS_TRACE is set) doesn't ImportError."""
    try:
        import types

        if "antenv.axon_hooks" in sys.modules:
            return
        import antenv

        mod = types.ModuleType("antenv.axon_hooks")
        mod._hook = None
        mod.set_axon_ntff_profile_hook = lambda h: setattr(mod, "_hook", h)
        mod.get_axon_ntff_profile_hook = lambda: mod._hook
        sys.modules["antenv.axon_hooks"] = mod
        antenv.axon_hooks = mod
        try:
            from trn_agent_boot.trn_boot import _ntff_profile_via_ctypes

            mod._hook = _ntff_profile_via_ctypes("/opt/axon/libaxon_pjrt.so")
        except Exception:
            pass
    except Exception:
        pass


_install_ntff_hook_shim()

# Problem shapes (hardcoded per spec)
B, T, D, Q = 64, 2048, 1024, 1024
N_CORES = 8
BL = B // N_CORES  # batches per core


def build(nc, BL=BL, T=T, D=D, Q=Q):
    """Emit the per-core kernel into `nc`. Returns nc."""
    f32 = mybir.dt.float32
    bf16 = mybir.dt.bfloat16
    fp16 = mybir.dt.float16
    fp8 = mybir.dt.float8e4
    AF = mybir.ActivationFunctionType
    ALU = mybir.AluOpType
    DR = mybir.MatmulPerfMode.DoubleRow

    P = 128
    TC = min(512, T)          # t-chunk size for the main GEMM
    DC = min(512, D)          # d-chunk size for the final output
    KD = D // P               # d contraction tiles
    ME = D // P               # e output tiles
    KQ = Q // P               # q contraction tiles
    NT = T // TC              # t chunks
    KT = T // P               # t contraction tiles (final sum)
    ND = D // DC              # output d chunks

    memT = nc.declare_dram_parameter("memT", [BL, NT, P, KD * TC], fp8, isOutput=False)
    memN = nc.declare_dram_parameter("memN", [BL, T, D], fp16, isOutput=False)
    wh = nc.declare_dram_parameter("Wh", [P, ME * D], fp8, isOutput=False)
    wq = nc.declare_dram_parameter("Wq", [P, KQ * D], bf16, isOutput=False)
    qryT = nc.declare_dram_parameter("qryT", [P, KQ * BL], bf16, isOutput=False)
    vT = nc.declare_dram_parameter("vT", [P, KD], f32, isOutput=False)
    out_ext = nc.declare_dram_parameter("out", [BL, D], f32, isOutput=True)

    with tile.TileContext(nc) as tc:
        from contextlib import ExitStack
        import functools

        with ExitStack() as ctx:
            const_pool = ctx.enter_context(tc.tile_pool(name="const", bufs=1))

            wh_sb = const_pool.tile([P, ME * D], fp8, tag="wh")
            v_sb = const_pool.tile([P, KD], f32, tag="v")
            qry_sb = const_pool.tile([P, KQ * BL], bf16, tag="qry")
            wq_sb = const_pool.tile([P, KQ * D], bf16, tag="wq")
            ones16_sb = const_pool.tile([P, 1], fp16, tag="ones16")
            nc.gpsimd.memset(ones16_sb[:], 1.0)
            onesw_sb = const_pool.tile([P, 32], bf16, tag="onesw")
            nc.gpsimd.memset(onesw_sb[:], 1.0)
            ones32_sb = const_pool.tile([P, 1], f32, tag="ones32")
            nc.gpsimd.memset(ones32_sb[:], 1.0)
            qT_sb = const_pool.tile([P, ME * BL], f32, tag="qT")  # col = m*BL+b

            mT_pool = ctx.enter_context(tc.tile_pool(name="mT", bufs=2))

            def emit_mT_load(b, tiles, eng=None):
                # col layout: n * (KD*TC) + k * TC + t ; each chunk is a
                # plain contiguous [128, KD*TC] copy (4 KB runs, fp8)
                mT_sb = mT_pool.tile([P, NT * KD * TC], fp8, tag="mT",
                                     name=f"mT{b}")
                for n in range(NT):
                    (eng or nc.gpsimd).dma_start(
                        mT_sb[:, n * KD * TC : (n + 1) * KD * TC],
                        memT[b, n],
                    )
                tiles[b] = mT_sb
                return tiles

            mT_tiles = {}
            mN_pool = ctx.enter_context(tc.tile_pool(name="mN", bufs=3))
            mN_tiles = {}

            def emit_mN_load(b):
                # memN is first needed by phase-3 (during batch b+1). All on
                # the sync ring: scalar/gpsimd sequencers are too busy to
                # issue kicks on time, and the ring sustains 4MB per batch.
                mN_sb = mN_pool.tile([P, KT * D], fp16, tag="mN", name=f"mN{b}")
                eng = nc.sync
                eng.dma_start(
                    mN_sb[:].rearrange("p (k d) -> p k d", k=KT),
                    memN[b].rearrange("(k p) d -> p k d", p=P),
                )
                mN_tiles[b] = mN_sb

            # Startup DMAs. Critical path: wh (1MB, m-major so m=0 lands
            # first) + memT[0] chunk 0 gate the first main matmul; qry+wq
            # (m-major, on the otherwise-idle scalar ring) gate the
            # interleaved qT blocks. Spread across rings.
            nc.scalar.dma_start(qry_sb[:], qryT[:])
            # wq in per-m chunks so qT(m) only waits on its own block's
            # completion semaphore, not the full 2MB transfer
            for m in range(ME):
                nc.scalar.dma_start(
                    wq_sb[:, m * KQ * P : (m + 1) * KQ * P],
                    wq[:, m * KQ * P : (m + 1) * KQ * P],
                )
            nc.sync.dma_start(wh_sb[:], wh[:])
            nc.sync.dma_start(v_sb[:], vT[:])
            # only batch 0's bulk tensors at startup — mT1/mN1 kicks are
            # deferred into batch 0 so the startup DMA rush doesn't starve
            # the critical mT0 chunks
            emit_mT_load(0, mT_tiles)       # gpsimd ring
            emit_mN_load(0)                 # sync ring

            # PE warm-up: dummy matmuls during the startup DMA window flip
            # the HAM clock gate to 8/8 before real work, so the first
            # main-GEMM matmuls run at 2.4 GHz
            with (
                tc.tile_pool(name="wup", bufs=1) as wu_pool,
                tc.tile_pool(name="wupp", bufs=1, space="PSUM") as wup_pool,
            ):
                wu_sb = wu_pool.tile([P, 512], bf16, tag="wu")
                nc.gpsimd.memset(wu_sb[:], 0.0)
                wu_ps = wup_pool.tile([32, 512], f32, tag="wups")
                for i in range(12):
                    nc.tensor.matmul(
                        wu_ps[:],
                        lhsT=onesw_sb[:],
                        rhs=wu_sb[:],
                        start=True,
                        stop=True,
                        skip_group_check=True,
                    )

            def emit_qT(m):
                # qT[:, m*BL:(m+1)*BL] = (Wq.T @ query).T block for m-tile m
                # (borrows a main-GEMM PSUM buffer; only used during batch 0)
                pq = ph_pool.tile([P, BL], f32, tag="ph", name=f"pq{m}")
                for k in range(KQ):
                    nc.tensor.matmul(
                        pq[:],
                        lhsT=wq_sb[
                            :, m * KQ * P + k * P : m * KQ * P + (k + 1) * P
                        ],
                        rhs=qry_sb[:, k * BL : (k + 1) * BL],
                        start=(k == 0),
                        stop=(k == KQ - 1),
                    )
                nc.scalar.copy(qT_sb[:, m * BL : (m + 1) * BL], pq[:])

            ph_pool = ctx.enter_context(tc.tile_pool(name="ph", bufs=4, space="PSUM"))
            pm_pool = ctx.enter_context(tc.tile_pool(name="pm", bufs=1, space="PSUM"))

            h_pool = ctx.enter_context(tc.tile_pool(name="h", bufs=3))
            hv_pool = ctx.enter_context(tc.tile_pool(name="hv", bufs=1))
            acc_pool = ctx.enter_context(tc.tile_pool(name="acc", bufs=2))
            accd_pool = ctx.enter_context(tc.tile_pool(name="accd", bufs=2))
            s_pool = ctx.enter_context(tc.tile_pool(name="s", bufs=2))

            # acc_d j-tiles handled by PE final-sum matmuls instead of
            # VectorE (the rest); balances PE vs Vector occupancy
            PE_J = 4

            def make_phase3(b, acc_hv, mN_sb, pe_all=False):
                # pe_all: run the whole weighted sum on the PE (no Vector
                # acc_d) — used for the second-to-last batch so VectorE is
                # free for the final batch's softmax chain
                pe_j = KT if pe_all else PE_J
                st = {}

                def emit_reduce_exp():
                    # s (transposed): sT[:, j] = acc_hv[:, j*128:(j+1)*128].T @ ones
                    sT_ps = pm_pool.tile([P, KT], f32, tag="sT", name=f"sT{b}")
                    for j in range(KT):
                        nc.tensor.matmul(
                            sT_ps[:, j : j + 1],
                            lhsT=acc_hv[:, j * P : (j + 1) * P],
                            rhs=ones16_sb[:, 0:1],
                            start=True,
                            stop=True,
                        )
                    # f32 exp output for the STT/TSM scalar columns (HW
                    # requires f32 scalars) + a tiny fp16 cast for the PE
                    # finals' lhsT
                    sT_exp = s_pool.tile([P, KT], f32, tag="sTe", name=f"sTe{b}")
                    partials = s_pool.tile([P, 1], f32, tag="par", name=f"par{b}")
                    nc.scalar.activation(
                        sT_exp[:], sT_ps[:], AF.Exp, accum_out=partials[:]
                    )
                    sT16 = s_pool.tile([P, KT], fp16, tag="sTe16",
                                       name=f"sTe16_{b}")
                    nc.scalar.copy(sT16[:, 0:pe_j], sT_exp[:, 0:pe_j])
                    st["sT_exp"] = sT_exp
                    st["sT16"] = sT16
                    st["partials"] = partials

                def emit_z():
                    rec = s_pool.tile([1, 1], f32, tag="rec", name=f"rec{b}")
                    zps = pm_pool.tile([1, 1], f32, tag="zps", name=f"zps{b}")
                    nc.tensor.matmul(
                        zps[0:1, 0:1],
                        lhsT=ones32_sb[:, 0:1],
                        rhs=st["partials"][:],
                        start=True,
                        stop=True,
                    )
                    nc.vector.reciprocal(rec[0:1, 0:1], zps[0:1, 0:1])
                    st["rec"] = rec

                def emit_pe_finals():
                    # first pe_j j-tiles of the weighted sum as PE matmuls,
                    # accumulating into the output PSUM row
                    ops = pm_pool.tile([1, D], f32, tag="ops", name=f"ops{b}")
                    st["ops"] = ops
                    for j in range(pe_j):
                        for n in range(ND):
                            nc.tensor.matmul(
                                ops[0:1, n * DC : (n + 1) * DC],
                                lhsT=st["sT16"][:, j : j + 1],
                                rhs=mN_sb[:, j * D + n * DC : j * D + (n + 1) * DC],
                                start=(j == 0),
                                stop=(pe_all and j == KT - 1),
                                skip_group_check=True,
                            )

                def emit_acc_part(j0, j1):
                    # acc_d[p, d] = sum_{j>=pe_j} memN_j[p, d] * s_exp[j*128+p]
                    if j0 == pe_j:
                        st["acc_d"] = accd_pool.tile(
                            [P, D], fp16, tag="accd", name=f"accd{b}"
                        )
                    acc_d = st["acc_d"]
                    for j in range(j0, j1):
                        if j == pe_j:
                            nc.vector.tensor_scalar_mul(
                                acc_d[:],
                                mN_sb[:, j * D : (j + 1) * D],
                                st["sT_exp"][:, j : j + 1],
                            )
                        else:
                            nc.vector.scalar_tensor_tensor(
                                acc_d[:],
                                mN_sb[:, j * D : (j + 1) * D],
                                st["sT_exp"][:, j : j + 1],
                                acc_d[:],
                                op0=ALU.mult,
                                op1=ALU.add,
                            )

                def emit_out():
                    out_row = s_pool.tile([1, D], f32, tag="orow",
                                          name=f"orow{b}", bufs=1)
                    ops = st["ops"]
                    if not pe_all:
                        for n in range(ND):
                            nc.tensor.matmul(
                                ops[0:1, n * DC : (n + 1) * DC],
                                lhsT=ones16_sb[:, 0:1],
                                rhs=st["acc_d"][:, n * DC : (n + 1) * DC],
                                start=False,
                                stop=True,
                                skip_group_check=True,
                            )
                    for n in range(ND):
                        nc.scalar.activation(
                            out_row[0:1, n * DC : (n + 1) * DC],
                            ops[0:1, n * DC : (n + 1) * DC],
                            AF.Copy,
                            scale=st["rec"][0:1, 0:1],
                        )
                    nc.sync.dma_start(out_ext[b : b + 1, :], out_row[:])

                parts = []
                if not pe_all:
                    nv = KT - pe_j
                    qq = max(1, nv // 4)
                    i = pe_j
                    while i < KT:
                        parts.append((i, min(i + qq, KT)))
                        i += qq
                # pe_finals late: by then mN(b) has long landed, so the PE
                # never waits on the memN stream
                return [emit_reduce_exp, emit_z] + [
                    functools.partial(emit_acc_part, a, c) for a, c in parts
                ] + [emit_pe_finals, emit_out]

            def emit_main_block(b, m, mT_sb, h_sb):
                # hidden.T m-tile: 4 n-chunks x (KD/2) DoubleRow fp8 matmuls
                wh3 = wh_sb[:].rearrange("p (m k c) -> p m k c", m=ME, k=KD)
                mT3 = mT_sb[:].rearrange("p (n k t) -> p n k t", n=NT, k=KD)
                for n in range(NT):
                    ph = ph_pool.tile([P, TC], f32, tag="ph",
                                      name=f"ph{b}_{m}_{n}")
                    for kk in range(KD // 2):
                        nc.tensor.matmul(
                            ph[:],
                            lhsT=wh3[:, m, 2 * kk : 2 * kk + 2, :],
                            rhs=mT3[:, n, 2 * kk : 2 * kk + 2, :],
                            start=(kk == 0),
                            stop=(kk == KD // 2 - 1),
                            perf_mode=DR,
                        )
                    nc.scalar.activation(
                        h_sb[:, n * TC : (n + 1) * TC], ph[:], AF.Tanh,
                        bias=qT_sb[:, m * BL + b : m * BL + b + 1],
                    )

            def emit_acc_hv(b, m, acc_hv, hv, h_sb):
                # v-weighted accumulation, all on VectorE: per-m multiply
                # via tensor_scalar (4x DVE mode, ~0.66us) + pairwise tree
                # adds via tensor_tensor (2x mode, ~1.14us). NOTE: GpSimd
                # shares SBUF ports with the DVE — offloading adds there
                # slows concurrent Vector ops ~4x, a net loss.
                if m < ME - 1:
                    # short-lived tiles share rotating tags for SBUF space
                    tag, nbufs = {
                        0: ("hv0", 1),
                        1: ("hvA", 2), 3: ("hvA", 2), 5: ("hvA", 2),
                        2: ("hvB", 2), 4: ("hvB", 2), 6: ("hvB", 2),
                    }[m]
                    hv[m] = hv_pool.tile([P, T], fp16, tag=tag,
                                         name=f"hv{b}_{m}", bufs=nbufs)
                    nc.vector.tensor_scalar_mul(
                        hv[m][:], h_sb[:], v_sb[:, m : m + 1])
                if m == 1:
                    nc.vector.tensor_tensor(
                        hv[0][:], hv[0][:], hv[1][:], op=ALU.add)
                elif m == 3:
                    nc.vector.tensor_tensor(
                        hv[2][:], hv[2][:], hv[3][:], op=ALU.add)
                    nc.vector.tensor_tensor(
                        hv[0][:], hv[0][:], hv[2][:], op=ALU.add)
                elif m == 5:
                    nc.vector.tensor_tensor(
                        hv[4][:], hv[4][:], hv[5][:], op=ALU.add)
                elif m == 6:
                    nc.vector.tensor_tensor(
                        hv[4][:], hv[4][:], hv[6][:], op=ALU.add)
                    nc.vector.tensor_tensor(
                        acc_hv[:], hv[0][:], hv[4][:], op=ALU.add)
                elif m == ME - 1:
                    # full-width multiply + in-place add: cheaper than 4
                    # chunked STTs; latency is fine since phase-3 only
                    # consumes acc_hv a few m-blocks into the next batch
                    hv7 = hv_pool.tile([P, T], fp16, tag="hvA",
                                       name=f"hv{b}_7", bufs=2)
                    nc.vector.tensor_scalar_mul(
                        hv7[:], h_sb[:], v_sb[:, m : m + 1])
                    nc.vector.tensor_tensor(
                        acc_hv[:], acc_hv[:], hv7[:], op=ALU.add)

            def emit_last_batch_tail(b, acc_hv, mN_sb, mT_sb, pending):
                # Final batch runs n-outer / m-inner: per t-chunk, all m
                # main blocks + tanh land first, then the chunk's STT chain,
                # sT columns, exp, and PE final-sum matmuls — so only the
                # very last chunk's (short) chain is exposed at the end.
                JPN = KT // NT  # j-tiles per n-chunk
                sT_ps = pm_pool.tile([P, KT], f32, tag="sT", name=f"sT{b}")
                sT16 = s_pool.tile([P, KT], fp16, tag="sTe16", name=f"sTe16_{b}")
                partials = s_pool.tile([P, NT], f32, tag="par", name=f"par{b}")
                ops = None  # allocated after prev batch's phase3 releases its slot
                wh3 = wh_sb[:].rearrange("p (m k c) -> p m k c", m=ME, k=KD)
                mT3 = mT_sb[:].rearrange("p (n k t) -> p n k t", n=NT, k=KD)
                for n in range(NT):
                    sl = slice(n * TC, (n + 1) * TC)
                    for m in range(ME):
                        ph = ph_pool.tile([P, TC], f32, tag="ph",
                                          name=f"ph{b}_{m}_{n}")
                        for kk in range(KD // 2):
                            nc.tensor.matmul(
                                ph[:],
                                lhsT=wh3[:, m, 2 * kk : 2 * kk + 2, :],
                                rhs=mT3[:, n, 2 * kk : 2 * kk + 2, :],
                                start=(kk == 0),
                                stop=(kk == KD // 2 - 1),
                                perf_mode=DR,
                            )
                        hc = h_pool.tile([P, TC], fp16, tag="hc",
                                         name=f"hc{b}_{m}_{n}", bufs=4)
                        nc.scalar.activation(
                            hc[:], ph[:], AF.Tanh,
                            bias=qT_sb[:, m * BL + b : m * BL + b + 1],
                        )
                        if m == 0:
                            nc.vector.tensor_scalar_mul(
                                acc_hv[:, sl], hc[:], v_sb[:, 0:1])
                        else:
                            nc.vector.scalar_tensor_tensor(
                                acc_hv[:, sl], hc[:], v_sb[:, m : m + 1],
                                acc_hv[:, sl], op0=ALU.mult, op1=ALU.add,
                            )
                    # drain prev batch's (all-PE) phase3 before this batch's
                    # own finals so the shared ops PSUM slot is allocated and
                    # released in program order
                    while pending:
                        pending.pop(0)()
                    if ops is None:
                        ops = pm_pool.tile([1, D], f32, tag="ops", name=f"ops{b}")
                    for j in range(n * JPN, (n + 1) * JPN):
                        nc.tensor.matmul(
                            sT_ps[:, j : j + 1],
                            lhsT=acc_hv[:, j * P : (j + 1) * P],
                            rhs=ones16_sb[:, 0:1],
                            start=True,
                            stop=True,
                        )
                    nc.scalar.activation(
                        sT16[:, n * JPN : (n + 1) * JPN],
                        sT_ps[:, n * JPN : (n + 1) * JPN],
                        AF.Exp,
                        accum_out=partials[:, n : n + 1],
                    )
                    for j in range(n * JPN, (n + 1) * JPN):
                        for d in range(ND):
                            nc.tensor.matmul(
                                ops[0:1, d * DC : (d + 1) * DC],
                                lhsT=sT16[:, j : j + 1],
                                rhs=mN_sb[:, j * D + d * DC : j * D + (d + 1) * DC],
                                start=(j == 0),
                                stop=(j == KT - 1),
                                skip_group_check=True,
                            )
                # Z = sum of the NT chunk-partials (each already summed over
                # partitions by exp's accum_out... no: accum_out sums along
                # the free axis per partition; partials[:, n] holds
                # per-partition sums, so one ones-matmul finishes the job)
                zps = pm_pool.tile([1, NT], f32, tag="zps", name=f"zps{b}")
                nc.tensor.matmul(
                    zps[0:1, 0:NT],
                    lhsT=ones32_sb[:, 0:1],
                    rhs=partials[:],
                    start=True,
                    stop=True,
                )
                # sum the NT chunk totals along the free axis via the
                # activation accumulator, then invert
                zscr = s_pool.tile([1, NT], f32, tag="zscr", name=f"zscr{b}")
                ztot = s_pool.tile([1, 1], f32, tag="zt", name=f"zt{b}")
                rec = s_pool.tile([1, 1], f32, tag="rec", name=f"rec{b}")
                nc.scalar.activation(
                    zscr[0:1, 0:NT], zps[0:1, 0:NT], AF.Copy,
                    accum_out=ztot[0:1, 0:1],
                )
                nc.vector.reciprocal(rec[0:1, 0:1], ztot[0:1, 0:1])
                out_row = s_pool.tile([1, D], f32, tag="orow",
                                      name=f"orow{b}", bufs=1)
                for n in range(ND):
                    nc.scalar.activation(
                        out_row[0:1, n * DC : (n + 1) * DC],
                        ops[0:1, n * DC : (n + 1) * DC],
                        AF.Copy,
                        scale=rec[0:1, 0:1],
                    )
                nc.sync.dma_start(out_ext[b : b + 1, :], out_row[:])

            # phase3 pieces of batch b run at checkpoints inside batch b+1
            CHECKPOINTS = {3, 4, 5, 6, 7}
            pending = []
            for b in range(BL):
                last = b == BL - 1
                if b not in mT_tiles:
                    emit_mT_load(b, mT_tiles)
                mT_sb = mT_tiles.pop(b)
                if b not in mN_tiles:
                    emit_mN_load(b)
                mN_sb = mN_tiles.pop(b)
                # prefetch next batch's memN a full batch ahead of first use
                # (batch 0 defers its kicks into the m loop instead)
                if b >= 1 and b + 1 < BL and (b + 1) not in mN_tiles:
                    emit_mN_load(b + 1)

                acc_hv = acc_pool.tile([P, T], fp16, tag="acc", name=f"acc{b}")
                hv = {}

                if last:
                    emit_last_batch_tail(b, acc_hv, mN_sb, mT_sb, pending)
                    pending = []
                    continue
                for m in range(ME):
                    if b == 0:
                        emit_qT(m)
                        if m == 3:
                            emit_mT_load(1, mT_tiles)
                        elif m == 5:
                            emit_mN_load(1)
                    h_sb = h_pool.tile([P, T], fp16, tag="h", name=f"h{b}_{m}")
                    emit_main_block(b, m, mT_sb, h_sb)
                    emit_acc_hv(b, m, acc_hv, hv, h_sb)
                    if m in CHECKPOINTS and pending:
                        pending.pop(0)()
                        while m == ME - 1 and pending:
                            pending.pop(0)()
                # prefetch batch b+2's memT now that this batch's slot frees
                if b + 2 < BL:
                    emit_mT_load(b + 2, mT_tiles)
                for fn in pending:
                    fn()
                pending = make_phase3(b, acc_hv, mN_sb, pe_all=(b == BL - 2))

    nc.compile()
    return nc


# ---------------------------------------------------------------------------
# Host side
# ---------------------------------------------------------------------------

_CACHED_NC = None


def _get_nc():
    global _CACHED_NC
    if _CACHED_NC is None:
        nc = bacc.Bacc("TRN2", target_bir_lowering=False, debug=False,
                       num_devices=N_CORES)
        _CACHED_NC = build(nc)
    return _CACHED_NC


def prep_in_maps(memory, query, Wh, Wq, v):
    """Shard + lay out inputs for the 8 cores (host-side transforms only)."""
    P = 128
    KQ = Q // P
    KD = D // P
    ME = D // P
    # m-major, k-pairs adjacent: col = m*KD*128 + k*128 + c
    Wh_b = np.ascontiguousarray(
        Wh.reshape(KD, P, ME, P).transpose(1, 2, 0, 3).reshape(P, ME * D)
        .astype(E4M3)
    )
    # m-major: col = m*KQ*128 + k*128 + c, so early m-blocks land first
    Wq_b = np.ascontiguousarray(
        Wq.reshape(KQ, P, ME, P).transpose(1, 2, 0, 3).reshape(P, KQ * D)
        .astype(BF16)
    )
    vT = np.ascontiguousarray(v[:, 0].reshape(KD, P).T.astype(np.float32))  # [128, KD]
    in_maps = []
    BLc = BL
    for c in range(N_CORES):
        sl = slice(c * BLc, (c + 1) * BLc)
        mem_c = memory[sl]
        # memT[b, n, p, k*TC+t] = mem[b, n*TC+t, k*128+p]
        TCc = min(512, T)
        NT = T // TCc
        memT_c = np.ascontiguousarray(
            mem_c.reshape(BLc, NT, TCc, KD, P)
            .transpose(0, 1, 4, 3, 2)
            .reshape(BLc, NT, P, KD * TCc)
            .astype(E4M3)
        )
        memN_c = np.ascontiguousarray(mem_c.astype(FP16))  # [BL, T, D]
        # qryT[p, k*BL+b] = query[b, k*128+p]  (exact SBUF layout)
        qryT_c = np.ascontiguousarray(
            query[sl].T.reshape(KQ, P, BLc).transpose(1, 0, 2).reshape(P, KQ * BLc)
            .astype(BF16)
        )
        in_maps.append(
            {
                "memT": memT_c,
                "memN": memN_c,
                "Wh": Wh_b,
                "Wq": Wq_b,
                "qryT": qryT_c,
                "vT": vT,
            }
        )
    return in_maps


def run(in_maps, trace=False, **kwargs):
    nc = _get_nc()
    return run_bass_kernel_spmd(
        nc, in_maps, list(range(N_CORES)), trace=trace, **kwargs
    )


def kernel(memory, query, Wh, Wq, v):
    in_maps = prep_in_maps(memory, query, Wh, Wq, v)
    # warm-up execution: first run on a cold device lands ~20% slower
    # (clock/DMA ramp); results are identical, so discard it
    run(in_maps)
    res = run(in_maps)
    out = np.concatenate([res.results[c]["out"] for c in range(N_CORES)], axis=0)
    return out.astype(np.float32)


# revision 49
# speedup vs baseline: 1.0233x; 1.0233x over previous
"""Trainium2 Bass kernel for nn_Attention_7584912245222.

Math (reference):
    hidden = tanh(memory @ Wh + (query @ Wq)[:, None, :])   # [B, T, D]
    s      = softmax(hidden @ v, axis=T)                    # [B, T]
    out    = einsum('btd,bt->bd', memory, s)                # [B, D]

Strategy: pure data-parallel over batch B=64 across 8 NeuronCores
(8 batches per core). Weights replicated. No collectives.

Device pipeline (per core, per batch b):
  - main GEMM in fp8 e4m3 with DoubleRow perf mode (2 contraction
    k-tiles per instruction, 2x PE throughput vs bf16). Computes
    hidden.T in [e(partitions), t(free)] orientation:
      lhsT = Wh[d,e] fp8 k-pair tiles (m-major layout, stationary)
      rhs  = mem[b].T fp8 tiles [d, t] (host pre-transposed, streamed)
    so the per-batch bias qvec[b][e] is a per-partition scalar, fused
    into the PSUM->SBUF tanh activation on ScalarE. The softmax
    averaging over T washes quantization noise out of the final
    weighted mean, keeping overall rel err ~1.8e-2 (< 2e-2 budget).
  - the bias qT = Wq.T @ query stays bf16 (its noise adds directly to
    the fp8 noise; bf16 keeps it negligible) and is computed per
    m-tile interleaved into batch 0 so the PE never waits for the Wq
    DMA.
  - v-weighting runs on the (otherwise idle) VectorE in fp16:
      acc_hv[p, t] += h_m[p, t] * v[m*128+p]   (scalar_tensor_tensor)
    and the partition reduction s[t] = sum_p acc_hv[p, t] is done with
    16 tiny PE matmuls producing s directly TRANSPOSED as [t, 1] cols.
  - softmax without max-subtraction (logits bounded, |s| < ~4).
  - final weighted sum on VectorE in fp16 (memN streamed as fp16):
      acc_d[p, d] += memN_j[p, d] * s_exp[j*128+p]
    followed by a 2-matmul partition reduction, scaled by 1/Z via the
    activation `scale` operand. The last batch uses PE matmuls
    pipelined per n-chunk instead, to shorten the kernel tail.

DMA rings: sync carries startup weights + memN(odd) + outputs,
gpsimd carries memT, vector carries memN(even) — keeps any single
ring under ~75 GB/s sustained so transfers never fall behind the PE.
"""

import sys

if "/opt/trn_rl_repo" not in sys.path:
    sys.path.insert(0, "/opt/trn_rl_repo")

import numpy as np
import ml_dtypes

import concourse.bass as bass
import concourse.tile as tile
from concourse import bacc, bass_isa, mybir
from concourse.bass_utils import run_bass_kernel_spmd

BF16 = ml_dtypes.bfloat16
FP16 = np.float16
E4M3 = ml_dtypes.float8_e4m3fn


def _install_ntff_hook_shim():
    """This image's antenv lacks axon_hooks; inject it so bass_utils'
    trace path (taken when BASS_TRACE is set) doesn't ImportError."""
    try:
        import types

        if "antenv.axon_hooks" in sys.modules:
            return
        import antenv

        mod = types.ModuleType("antenv.axon_hooks")
        mod._hook = None
        mod.set_axon_ntff_profile_hook = lambda h: setattr(mod, "_hook", h)
        mod.get_axon_ntff_profile_hook = lambda: mod._hook
        sys.modules["antenv.axon_hooks"] = mod
        antenv.axon_hooks = mod
        try:
            from trn_agent_boot.trn_boot import _ntff_profile_via_ctypes

            mod._hook = _ntff_profile_via_ctypes("/opt/axon/libaxon_pjrt.so")
        except Exception:
            pass
    except Exception:
        pass


_install_ntff_hook_shim()

# Problem shapes (hardcoded per spec)
B, T, D, Q = 64, 2048, 1024, 1024
N_CORES = 8
BL = B // N_CORES  # batches per core


def build(nc, BL=BL, T=T, D=D, Q=Q):
    """Emit the per-core kernel into `nc`. Returns nc."""
    f32 = mybir.dt.float32
    bf16 = mybir.dt.bfloat16
    fp16 = mybir.dt.float16
    fp8 = mybir.dt.float8e4
    AF = mybir.ActivationFunctionType
    ALU = mybir.AluOpType
    DR = mybir.MatmulPerfMode.DoubleRow

    P = 128
    TC = min(512, T)          # t-chunk size for the main GEMM
    DC = min(512, D)          # d-chunk size for the final output
    KD = D // P               # d contraction tiles
    ME = D // P               # e output tiles
    KQ = Q // P               # q contraction tiles
    NT = T // TC              # t chunks
    KT = T // P               # t contraction tiles (final sum)
    ND = D // DC              # output d chunks

    memT = nc.declare_dram_parameter("memT", [BL, NT, P, KD * TC], fp8, isOutput=False)
    memN = nc.declare_dram_parameter("memN", [BL, T, D], fp16, isOutput=False)
    wh = nc.declare_dram_parameter("Wh", [P, ME * D], fp8, isOutput=False)
    wq = nc.declare_dram_parameter("Wq", [P, KQ * D], bf16, isOutput=False)
    qryT = nc.declare_dram_parameter("qryT", [P, KQ * BL], bf16, isOutput=False)
    vT = nc.declare_dram_parameter("vT", [P, KD], f32, isOutput=False)
    out_ext = nc.declare_dram_parameter("out", [BL, D], f32, isOutput=True)

    with tile.TileContext(nc) as tc:
        from contextlib import ExitStack
        import functools

        with ExitStack() as ctx:
            const_pool = ctx.enter_context(tc.tile_pool(name="const", bufs=1))

            wh_sb = const_pool.tile([P, ME * D], fp8, tag="wh")
            v_sb = const_pool.tile([P, KD], f32, tag="v")
            qry_sb = const_pool.tile([P, KQ * BL], bf16, tag="qry")
            wq_sb = const_pool.tile([P, KQ * D], bf16, tag="wq")
            ones16_sb = const_pool.tile([P, 1], fp16, tag="ones16")
            nc.gpsimd.memset(ones16_sb[:], 1.0)
            onesw_sb = const_pool.tile([P, 32], bf16, tag="onesw")
            nc.gpsimd.memset(onesw_sb[:], 1.0)
            ones32_sb = const_pool.tile([P, 1], f32, tag="ones32")
            nc.gpsimd.memset(ones32_sb[:], 1.0)
            qT_sb = const_pool.tile([P, ME * BL], f32, tag="qT")  # col = m*BL+b

            mT_pool = ctx.enter_context(tc.tile_pool(name="mT", bufs=2))

            def emit_mT_load(b, tiles, eng=None):
                # col layout: n * (KD*TC) + k * TC + t ; each chunk is a
                # plain contiguous [128, KD*TC] copy (4 KB runs, fp8)
                mT_sb = mT_pool.tile([P, NT * KD * TC], fp8, tag="mT",
                                     name=f"mT{b}")
                for n in range(NT):
                    (eng or nc.gpsimd).dma_start(
                        mT_sb[:, n * KD * TC : (n + 1) * KD * TC],
                        memT[b, n],
                    )
                tiles[b] = mT_sb
                return tiles

            mT_tiles = {}
            mN_pool = ctx.enter_context(tc.tile_pool(name="mN", bufs=3))
            mN_tiles = {}

            def emit_mN_load(b):
                # memN is first needed by phase-3 (during batch b+1). All on
                # the sync ring: scalar/gpsimd sequencers are too busy to
                # issue kicks on time, and the ring sustains 4MB per batch.
                mN_sb = mN_pool.tile([P, KT * D], fp16, tag="mN", name=f"mN{b}")
                eng = nc.sync
                eng.dma_start(
                    mN_sb[:].rearrange("p (k d) -> p k d", k=KT),
                    memN[b].rearrange("(k p) d -> p k d", p=P),
                )
                mN_tiles[b] = mN_sb

            # Startup DMAs. Critical path: wh (1MB, m-major so m=0 lands
            # first) + memT[0] chunk 0 gate the first main matmul; qry+wq
            # (m-major, on the otherwise-idle scalar ring) gate the
            # interleaved qT blocks. Spread across rings.
            nc.scalar.dma_start(qry_sb[:], qryT[:])
            # wq in per-m chunks so qT(m) only waits on its own block's
            # completion semaphore, not the full 2MB transfer
            for m in range(ME):
                nc.scalar.dma_start(
                    wq_sb[:, m * KQ * P : (m + 1) * KQ * P],
                    wq[:, m * KQ * P : (m + 1) * KQ * P],
                )
            nc.sync.dma_start(wh_sb[:], wh[:])
            nc.sync.dma_start(v_sb[:], vT[:])
            # only batch 0's bulk tensors at startup — mT1/mN1 kicks are
            # deferred into batch 0 so the startup DMA rush doesn't starve
            # the critical mT0 chunks
            emit_mT_load(0, mT_tiles)       # gpsimd ring
            emit_mN_load(0)                 # sync ring

            # PE warm-up: dummy matmuls during the startup DMA window flip
            # the HAM clock gate to 8/8 before real work, so the first
            # main-GEMM matmuls run at 2.4 GHz
            with (
                tc.tile_pool(name="wup", bufs=1) as wu_pool,
                tc.tile_pool(name="wupp", bufs=1, space="PSUM") as wup_pool,
            ):
                wu_sb = wu_pool.tile([P, 512], bf16, tag="wu")
                nc.gpsimd.memset(wu_sb[:], 0.0)
                wu_ps = wup_pool.tile([32, 512], f32, tag="wups")
                for i in range(12):
                    nc.tensor.matmul(
                        wu_ps[:],
                        lhsT=onesw_sb[:],
                        rhs=wu_sb[:],
                        start=True,
                        stop=True,
                        skip_group_check=True,
                    )

            def emit_qT(m):
                # qT[:, m*BL:(m+1)*BL] = (Wq.T @ query).T block for m-tile m
                # (borrows a main-GEMM PSUM buffer; only used during batch 0)
                pq = ph_pool.tile([P, BL], f32, tag="ph", name=f"pq{m}")
                for k in range(KQ):
                    nc.tensor.matmul(
                        pq[:],
                        lhsT=wq_sb[
                            :, m * KQ * P + k * P : m * KQ * P + (k + 1) * P
                        ],
                        rhs=qry_sb[:, k * BL : (k + 1) * BL],
                        start=(k == 0),
                        stop=(k == KQ - 1),
                    )
                nc.scalar.copy(qT_sb[:, m * BL : (m + 1) * BL], pq[:])

            ph_pool = ctx.enter_context(tc.tile_pool(name="ph", bufs=4, space="PSUM"))
            pm_pool = ctx.enter_context(tc.tile_pool(name="pm", bufs=1, space="PSUM"))

            h_pool = ctx.enter_context(tc.tile_pool(name="h", bufs=3))
            hv_pool = ctx.enter_context(tc.tile_pool(name="hv", bufs=1))
            acc_pool = ctx.enter_context(tc.tile_pool(name="acc", bufs=2))
            accd_pool = ctx.enter_context(tc.tile_pool(name="accd", bufs=2))
            s_pool = ctx.enter_context(tc.tile_pool(name="s", bufs=2))

            # acc_d j-tiles handled by PE final-sum matmuls instead of
            # VectorE (the rest); balances PE vs Vector occupancy
            PE_J = 2

            def make_phase3(b, acc_hv, mN_sb, pe_all=False):
                # pe_all: run the whole weighted sum on the PE (no Vector
                # acc_d) — used for the second-to-last batch so VectorE is
                # free for the final batch's softmax chain
                pe_j = KT if pe_all else PE_J
                st = {}

                def emit_reduce_exp():
                    # s (transposed): sT[:, j] = acc_hv[:, j*128:(j+1)*128].T @ ones
                    sT_ps = pm_pool.tile([P, KT], f32, tag="sT", name=f"sT{b}")
                    for j in range(KT):
                        nc.tensor.matmul(
                            sT_ps[:, j : j + 1],
                            lhsT=acc_hv[:, j * P : (j + 1) * P],
                            rhs=ones16_sb[:, 0:1],
                            start=True,
                            stop=True,
                        )
                    # f32 exp output for the STT/TSM scalar columns (HW
                    # requires f32 scalars) + a tiny fp16 cast for the PE
                    # finals' lhsT
                    sT_exp = s_pool.tile([P, KT], f32, tag="sTe", name=f"sTe{b}")
                    partials = s_pool.tile([P, 1], f32, tag="par", name=f"par{b}")
                    nc.scalar.activation(
                        sT_exp[:], sT_ps[:], AF.Exp, accum_out=partials[:]
                    )
                    sT16 = s_pool.tile([P, KT], fp16, tag="sTe16",
                                       name=f"sTe16_{b}")
                    nc.scalar.copy(sT16[:, 0:pe_j], sT_exp[:, 0:pe_j])
                    st["sT_exp"] = sT_exp
                    st["sT16"] = sT16
                    st["partials"] = partials

                def emit_z():
                    rec = s_pool.tile([1, 1], f32, tag="rec", name=f"rec{b}")
                    zps = pm_pool.tile([1, 1], f32, tag="zps", name=f"zps{b}")
                    nc.tensor.matmul(
                        zps[0:1, 0:1],
                        lhsT=ones32_sb[:, 0:1],
                        rhs=st["partials"][:],
                        start=True,
                        stop=True,
                    )
                    nc.vector.reciprocal(rec[0:1, 0:1], zps[0:1, 0:1])
                    st["rec"] = rec

                def emit_pe_finals():
                    # first pe_j j-tiles of the weighted sum as PE matmuls,
                    # accumulating into the output PSUM row
                    ops = pm_pool.tile([1, D], f32, tag="ops", name=f"ops{b}")
                    st["ops"] = ops
                    for j in range(pe_j):
                        for n in range(ND):
                            nc.tensor.matmul(
                                ops[0:1, n * DC : (n + 1) * DC],
                                lhsT=st["sT16"][:, j : j + 1],
                                rhs=mN_sb[:, j * D + n * DC : j * D + (n + 1) * DC],
                                start=(j == 0),
                                stop=(pe_all and j == KT - 1),
                                skip_group_check=True,
                            )

                def emit_acc_part(j0, j1):
                    # acc_d[p, d] = sum_{j>=pe_j} memN_j[p, d] * s_exp[j*128+p]
                    if j0 == pe_j:
                        st["acc_d"] = accd_pool.tile(
                            [P, D], fp16, tag="accd", name=f"accd{b}"
                        )
                    acc_d = st["acc_d"]
                    for j in range(j0, j1):
                        if j == pe_j:
                            nc.vector.tensor_scalar_mul(
                                acc_d[:],
                                mN_sb[:, j * D : (j + 1) * D],
                                st["sT_exp"][:, j : j + 1],
                            )
                        else:
                            nc.vector.scalar_tensor_tensor(
                                acc_d[:],
                                mN_sb[:, j * D : (j + 1) * D],
                                st["sT_exp"][:, j : j + 1],
                                acc_d[:],
                                op0=ALU.mult,
                                op1=ALU.add,
                            )

                def emit_out():
                    out_row = s_pool.tile([1, D], f32, tag="orow",
                                          name=f"orow{b}", bufs=1)
                    ops = st["ops"]
                    if not pe_all:
                        for n in range(ND):
                            nc.tensor.matmul(
                                ops[0:1, n * DC : (n + 1) * DC],
                                lhsT=ones16_sb[:, 0:1],
                                rhs=st["acc_d"][:, n * DC : (n + 1) * DC],
                                start=False,
                                stop=True,
                                skip_group_check=True,
                            )
                    for n in range(ND):
                        nc.scalar.activation(
                            out_row[0:1, n * DC : (n + 1) * DC],
                            ops[0:1, n * DC : (n + 1) * DC],
                            AF.Copy,
                            scale=st["rec"][0:1, 0:1],
                        )
                    nc.sync.dma_start(out_ext[b : b + 1, :], out_row[:])

                parts = []
                if not pe_all:
                    nv = KT - pe_j
                    qq = max(1, nv // 4)
                    i = pe_j
                    while i < KT:
                        parts.append((i, min(i + qq, KT)))
                        i += qq
                # pe_finals late: by then mN(b) has long landed, so the PE
                # never waits on the memN stream
                return [emit_reduce_exp, emit_z] + [
                    functools.partial(emit_acc_part, a, c) for a, c in parts
                ] + [emit_pe_finals, emit_out]

            def emit_main_block(b, m, mT_sb, h_sb):
                # hidden.T m-tile: 4 n-chunks x (KD/2) DoubleRow fp8 matmuls
                wh3 = wh_sb[:].rearrange("p (m k c) -> p m k c", m=ME, k=KD)
                mT3 = mT_sb[:].rearrange("p (n k t) -> p n k t", n=NT, k=KD)
                for n in range(NT):
                    ph = ph_pool.tile([P, TC], f32, tag="ph",
                                      name=f"ph{b}_{m}_{n}")
                    for kk in range(KD // 2):
                        nc.tensor.matmul(
                            ph[:],
                            lhsT=wh3[:, m, 2 * kk : 2 * kk + 2, :],
                            rhs=mT3[:, n, 2 * kk : 2 * kk + 2, :],
                            start=(kk == 0),
                            stop=(kk == KD // 2 - 1),
                            perf_mode=DR,
                        )
                    nc.scalar.activation(
                        h_sb[:, n * TC : (n + 1) * TC], ph[:], AF.Tanh,
                        bias=qT_sb[:, m * BL + b : m * BL + b + 1],
                    )

            def emit_acc_hv(b, m, acc_hv, hv, h_sb):
                # v-weighted accumulation, all on VectorE: per-m multiply
                # via tensor_scalar (4x DVE mode, ~0.66us) + pairwise tree
                # adds via tensor_tensor (2x mode, ~1.14us). NOTE: GpSimd
                # shares SBUF ports with the DVE — offloading adds there
                # slows concurrent Vector ops ~4x, a net loss.
                if m < ME - 1:
                    # short-lived tiles share rotating tags for SBUF space
                    tag, nbufs = {
                        0: ("hv0", 1),
                        1: ("hvA", 2), 3: ("hvA", 2), 5: ("hvA", 2),
                        2: ("hvB", 2), 4: ("hvB", 2), 6: ("hvB", 2),
                    }[m]
                    hv[m] = hv_pool.tile([P, T], fp16, tag=tag,
                                         name=f"hv{b}_{m}", bufs=nbufs)
                    nc.vector.tensor_scalar_mul(
                        hv[m][:], h_sb[:], v_sb[:, m : m + 1])
                if m == 1:
                    nc.vector.tensor_tensor(
                        hv[0][:], hv[0][:], hv[1][:], op=ALU.add)
                elif m == 3:
                    nc.vector.tensor_tensor(
                        hv[2][:], hv[2][:], hv[3][:], op=ALU.add)
                    nc.vector.tensor_tensor(
                        hv[0][:], hv[0][:], hv[2][:], op=ALU.add)
                elif m == 5:
                    nc.vector.tensor_tensor(
                        hv[4][:], hv[4][:], hv[5][:], op=ALU.add)
                elif m == 6:
                    nc.vector.tensor_tensor(
                        hv[4][:], hv[4][:], hv[6][:], op=ALU.add)
                    nc.vector.tensor_tensor(
                        acc_hv[:], hv[0][:], hv[4][:], op=ALU.add)
                elif m == ME - 1:
                    # full-width multiply + in-place add: cheaper than 4
                    # chunked STTs; latency is fine since phase-3 only
                    # consumes acc_hv a few m-blocks into the next batch
                    hv7 = hv_pool.tile([P, T], fp16, tag="hvA",
                                       name=f"hv{b}_7", bufs=2)
                    nc.vector.tensor_scalar_mul(
                        hv7[:], h_sb[:], v_sb[:, m : m + 1])
                    nc.vector.tensor_tensor(
                        acc_hv[:], acc_hv[:], hv7[:], op=ALU.add)

            def emit_last_batch_tail(b, acc_hv, mN_sb, mT_sb, pending):
                # Final batch runs n-outer / m-inner: per t-chunk, all m
                # main blocks + tanh land first, then the chunk's STT chain,
                # sT columns, exp, and PE final-sum matmuls — so only the
                # very last chunk's (short) chain is exposed at the end.
                JPN = KT // NT  # j-tiles per n-chunk
                sT_ps = pm_pool.tile([P, KT], f32, tag="sT", name=f"sT{b}")
                sT16 = s_pool.tile([P, KT], fp16, tag="sTe16", name=f"sTe16_{b}")
                partials = s_pool.tile([P, NT], f32, tag="par", name=f"par{b}")
                ops = None  # allocated after prev batch's phase3 releases its slot
                wh3 = wh_sb[:].rearrange("p (m k c) -> p m k c", m=ME, k=KD)
                mT3 = mT_sb[:].rearrange("p (n k t) -> p n k t", n=NT, k=KD)
                for n in range(NT):
                    sl = slice(n * TC, (n + 1) * TC)
                    for m in range(ME):
                        ph = ph_pool.tile([P, TC], f32, tag="ph",
                                          name=f"ph{b}_{m}_{n}")
                        for kk in range(KD // 2):
                            nc.tensor.matmul(
                                ph[:],
                                lhsT=wh3[:, m, 2 * kk : 2 * kk + 2, :],
                                rhs=mT3[:, n, 2 * kk : 2 * kk + 2, :],
                                start=(kk == 0),
                                stop=(kk == KD // 2 - 1),
                                perf_mode=DR,
                            )
                        hc = h_pool.tile([P, TC], fp16, tag="hc",
                                         name=f"hc{b}_{m}_{n}", bufs=4)
                        nc.scalar.activation(
                            hc[:], ph[:], AF.Tanh,
                            bias=qT_sb[:, m * BL + b : m * BL + b + 1],
                        )
                        if m == 0:
                            nc.vector.tensor_scalar_mul(
                                acc_hv[:, sl], hc[:], v_sb[:, 0:1])
                        else:
                            nc.vector.scalar_tensor_tensor(
                                acc_hv[:, sl], hc[:], v_sb[:, m : m + 1],
                                acc_hv[:, sl], op0=ALU.mult, op1=ALU.add,
                            )
                    # drain prev batch's (all-PE) phase3 before this batch's
                    # own finals so the shared ops PSUM slot is allocated and
                    # released in program order
                    while pending:
                        pending.pop(0)()
                    if ops is None:
                        ops = pm_pool.tile([1, D], f32, tag="ops", name=f"ops{b}")
                    for j in range(n * JPN, (n + 1) * JPN):
                        nc.tensor.matmul(
                            sT_ps[:, j : j + 1],
                            lhsT=acc_hv[:, j * P : (j + 1) * P],
                            rhs=ones16_sb[:, 0:1],
                            start=True,
                            stop=True,
                        )
                    nc.scalar.activation(
                        sT16[:, n * JPN : (n + 1) * JPN],
                        sT_ps[:, n * JPN : (n + 1) * JPN],
                        AF.Exp,
                        accum_out=partials[:, n : n + 1],
                    )
                    for j in range(n * JPN, (n + 1) * JPN):
                        for d in range(ND):
                            nc.tensor.matmul(
                                ops[0:1, d * DC : (d + 1) * DC],
                                lhsT=sT16[:, j : j + 1],
                                rhs=mN_sb[:, j * D + d * DC : j * D + (d + 1) * DC],
                                start=(j == 0),
                                stop=(j == KT - 1),
                                skip_group_check=True,
                            )
                # Z = sum of the NT chunk-partials (each already summed over
                # partitions by exp's accum_out... no: accum_out sums along
                # the free axis per partition; partials[:, n] holds
                # per-partition sums, so one ones-matmul finishes the job)
                zps = pm_pool.tile([1, NT], f32, tag="zps", name=f"zps{b}")
                nc.tensor.matmul(
                    zps[0:1, 0:NT],
                    lhsT=ones32_sb[:, 0:1],
                    rhs=partials[:],
                    start=True,
                    stop=True,
                )
                # sum the NT chunk totals along the free axis via the
                # activation accumulator, then invert
                zscr = s_pool.tile([1, NT], f32, tag="zscr", name=f"zscr{b}")
                ztot = s_pool.tile([1, 1], f32, tag="zt", name=f"zt{b}")
                rec = s_pool.tile([1, 1], f32, tag="rec", name=f"rec{b}")
                nc.scalar.activation(
                    zscr[0:1, 0:NT], zps[0:1, 0:NT], AF.Copy,
                    accum_out=ztot[0:1, 0:1],
                )
                nc.vector.reciprocal(rec[0:1, 0:1], ztot[0:1, 0:1])
                out_row = s_pool.tile([1, D], f32, tag="orow",
                                      name=f"orow{b}", bufs=1)
                for n in range(ND):
                    nc.scalar.activation(
                        out_row[0:1, n * DC : (n + 1) * DC],
                        ops[0:1, n * DC : (n + 1) * DC],
                        AF.Copy,
                        scale=rec[0:1, 0:1],
                    )
                nc.sync.dma_start(out_ext[b : b + 1, :], out_row[:])

            # phase3 pieces of batch b run at checkpoints inside batch b+1
            CHECKPOINTS = {3, 4, 5, 6, 7}
            pending = []
            for b in range(BL):
                last = b == BL - 1
                if b not in mT_tiles:
                    emit_mT_load(b, mT_tiles)
                mT_sb = mT_tiles.pop(b)
                if b not in mN_tiles:
                    emit_mN_load(b)
                mN_sb = mN_tiles.pop(b)
                # prefetch next batch's memN a full batch ahead of first use
                # (batch 0 defers its kicks into the m loop instead)
                if b >= 1 and b + 1 < BL and (b + 1) not in mN_tiles:
                    emit_mN_load(b + 1)

                acc_hv = acc_pool.tile([P, T], fp16, tag="acc", name=f"acc{b}")
                hv = {}

                if last:
                    emit_last_batch_tail(b, acc_hv, mN_sb, mT_sb, pending)
                    pending = []
                    continue
                for m in range(ME):
                    if b == 0:
                        emit_qT(m)
                        if m == 3:
                            emit_mT_load(1, mT_tiles)
                        elif m == 5:
                            emit_mN_load(1)
                    h_sb = h_pool.tile([P, T], fp16, tag="h", name=f"h{b}_{m}")
                    emit_main_block(b, m, mT_sb, h_sb)
                    emit_acc_hv(b, m, acc_hv, hv, h_sb)
                    if m in CHECKPOINTS and pending:
                        pending.pop(0)()
                        while m == ME - 1 and pending:
                            pending.pop(0)()
                # prefetch batch b+2's memT now that this batch's slot frees
                if b + 2 < BL:
                    emit_mT_load(b + 2, mT_tiles)
                for fn in pending:
                    fn()
                pending = make_phase3(b, acc_hv, mN_sb, pe_all=(b == BL - 2))

    nc.compile()
    return nc


# ---------------------------------------------------------------------------
# Host side
# ---------------------------------------------------------------------------

_CACHED_NC = None


def _get_nc():
    global _CACHED_NC
    if _CACHED_NC is None:
        nc = bacc.Bacc("TRN2", target_bir_lowering=False, debug=False,
                       num_devices=N_CORES)
        _CACHED_NC = build(nc)
    return _CACHED_NC


def prep_in_maps(memory, query, Wh, Wq, v):
    """Shard + lay out inputs for the 8 cores (host-side transforms only)."""
    P = 128
    KQ = Q // P
    KD = D // P
    ME = D // P
    # m-major, k-pairs adjacent: col = m*KD*128 + k*128 + c
    Wh_b = np.ascontiguousarray(
        Wh.reshape(KD, P, ME, P).transpose(1, 2, 0, 3).reshape(P, ME * D)
        .astype(E4M3)
    )
    # m-major: col = m*KQ*128 + k*128 + c, so early m-blocks land first
    Wq_b = np.ascontiguousarray(
        Wq.reshape(KQ, P, ME, P).transpose(1, 2, 0, 3).reshape(P, KQ * D)
        .astype(BF16)
    )
    vT = np.ascontiguousarray(v[:, 0].reshape(KD, P).T.astype(np.float32))  # [128, KD]
    in_maps = []
    BLc = BL
    for c in range(N_CORES):
        sl = slice(c * BLc, (c + 1) * BLc)
        mem_c = memory[sl]
        # memT[b, n, p, k*TC+t] = mem[b, n*TC+t, k*128+p]
        TCc = min(512, T)
        NT = T // TCc
        memT_c = np.ascontiguousarray(
            mem_c.reshape(BLc, NT, TCc, KD, P)
            .transpose(0, 1, 4, 3, 2)
            .reshape(BLc, NT, P, KD * TCc)
            .astype(E4M3)
        )
        memN_c = np.ascontiguousarray(mem_c.astype(FP16))  # [BL, T, D]
        # qryT[p, k*BL+b] = query[b, k*128+p]  (exact SBUF layout)
        qryT_c = np.ascontiguousarray(
            query[sl].T.reshape(KQ, P, BLc).transpose(1, 0, 2).reshape(P, KQ * BLc)
            .astype(BF16)
        )
        in_maps.append(
            {
                "memT": memT_c,
                "memN": memN_c,
                "Wh": Wh_b,
                "Wq": Wq_b,
                "qryT": qryT_c,
                "vT": vT,
            }
        )
    return in_maps


def run(in_maps, trace=False, **kwargs):
    nc = _get_nc()
    return run_bass_kernel_spmd(
        nc, in_maps, list(range(N_CORES)), trace=trace, **kwargs
    )


def kernel(memory, query, Wh, Wq, v):
    in_maps = prep_in_maps(memory, query, Wh, Wq, v)
    # warm-up execution: first run on a cold device lands ~20% slower
    # (clock/DMA ramp); results are identical, so discard it
    run(in_maps)
    res = run(in_maps)
    out = np.concatenate([res.results[c]["out"] for c in range(N_CORES)], axis=0)
    return out.astype(np.float32)
